# revision 1
# baseline (speedup 1.0000x reference)
"""Trainium2 Bass kernel for the hypernet-Conv3D module.

Strategy (data-parallel over batch, one sample per NeuronCore):
  - The tiny hypernet MLP (~2 MFLOP vs 58 GFLOP for the conv) runs on the
    host in fp32 numpy; it produces per-sample conv weights [32,16,3,3,3] and
    biases [32], repacked into matmul-ready block-Toeplitz layouts (bf16).
  - The 3D conv runs on device as an implicit GEMM ("Toeplitz-D"):
      * x (host-transposed to [d, cin, h, w] bf16) is processed in 16 windows
        of 4 output d-planes; each window holds 6 input planes (halo 1) in
        SBUF as [96 = 6 planes x 16 cin, 66x66 zero-padded hw] - one
        contiguous-source DMA per window.
      * PSUM tile [128 = 4 outplanes x 32 cout, N cols]; 9 accumulating bf16
        matmuls (one per (kh,kw) offset, applied as a free-dim shift of the
        rhs AP). kd offsets ride inside the block-Toeplitz lhsT [96, 128].
      * Chunks are row-aligned (7 padded rows = 462 cols, final 1-row chunk)
        so the ScalarE PSUM->SBUF evacuation (bias fused) also compacts the
        66-wide padded rows to 64-wide valid rows.
      * One contiguous 2 MB DMA per window writes [4 planes x 32 cout] back.
"""

import numpy as np
import ml_dtypes

import concourse.bacc as bacc
import concourse.mybir as mybir
from concourse.tile import TileContext
from concourse.bass_utils import run_bass_kernel_spmd

B, CIN, COUT, K = 8, 16, 32, 3
D = H = W = 64
NUM_W = CIN * COUT * K**3  # 13824

PW = W + 2          # 66
P2 = PW * PW        # 4356 padded plane
MARGIN = PW + 1     # 67 >= max |(kh-1)*66 + (kw-1)|
XFREE = P2 + 2 * MARGIN  # 4490
GD = 4              # output d-planes per window
NWIN = D // GD      # 16
NPL = GD + 2        # input planes per window
N_CORES = 8
# (start_padded_row, n_rows) chunks covering padded rows 1..64
ROW_CHUNKS = [(1 + 7 * i, 7) for i in range(9)] + [(64, 1)]

f32 = mybir.dt.float32
bf16 = mybir.dt.bfloat16

# matmul datapath: "bf16" (half bytes, ~2e-3 rel err) or "f32r" (fp32
# storage, PE fast-fp32 mode, near-fp32 accuracy)
import os as _os
MM_MODE = _os.environ.get("CONV_MM_MODE", "bf16")
if MM_MODE == "bf16":
    DT = bf16
    NPDT = ml_dtypes.bfloat16
    MM_CAST = None
else:
    # float32r end-to-end: walrus requires matmul operands to be f32r-typed
    # at the producer, so the DRAM tensors and SBUF tiles carry the dtype.
    DT = mybir.dt.float32r
    NPDT = np.float32
    MM_CAST = None


# ---------------------------------------------------------------- host side

def _host_hypernet(inputs):
    f = np.asarray(inputs["features"], np.float32)
    fc0_w = np.asarray(inputs["fc0_w"], np.float32)
    fc0_b = np.asarray(inputs["fc0_b"], np.float32)
    fc1_w = np.asarray(inputs["fc1_w"], np.float32)
    fc1_b = np.asarray(inputs["fc1_b"], np.float32)
    a0 = np.float32(np.asarray(inputs["a0"]).reshape(-1)[0])
    a1 = np.float32(np.asarray(inputs["a1"]).reshape(-1)[0])
    wg_w = np.asarray(inputs["wg_w"], np.float32)
    wg_b = np.asarray(inputs["wg_b"], np.float32)
    h = f @ fc0_w.T + fc0_b
    h = np.where(h >= 0, h, a0 * h)
    h = h @ fc1_w.T + fc1_b
    h = np.where(h >= 0, h, a1 * h)
    params = h @ wg_w.T + wg_b
    w = params[:, :NUM_W].reshape(B, COUT, CIN, K, K, K).astype(np.float32)
    bias = params[:, NUM_W:].astype(np.float32)
    return w, bias


def _build_wmat(w):
    """w: [32,16,3,3,3] -> [96, 9*128] block-Toeplitz lhsT bank (bf16).

    Column block i = 3*kh + kw holds lhsT_i with
      lhsT_i[16*pl + cin, 32*j + c] = w[c, cin, pl - j, kh, kw]  (0 <= pl-j <= 2)
    """
    wmat = np.zeros((9, 96, 128), np.float32)
    wt = np.transpose(w, (3, 4, 1, 0, 2))  # [kh, kw, cin, cout, kd]
    for kh in range(3):
        for kw in range(3):
            i = 3 * kh + kw
            for j in range(GD):
                for kd in range(3):
                    pl = j + kd
                    wmat[i, 16 * pl:16 * pl + 16, 32 * j:32 * j + 32] = \
                        wt[kh, kw, :, :, kd]
    return np.ascontiguousarray(
        wmat.transpose(1, 0, 2).reshape(96, 9 * 128).astype(NPDT))


# -------------------------------------------------------------- device side

def _conv_body(tc, xt_d, wm_d, bias_d, y_d):
    nc = tc.nc
    with (
        tc.tile_pool(name="const", bufs=1) as cpool,
        tc.tile_pool(name="xw", bufs=1) as xpool,
        tc.tile_pool(name="osb", bufs=3) as opool,
        tc.tile_pool(name="ps", bufs=6, space="PSUM") as pspool,
    ):
        wsb = cpool.tile([96, 9 * 128], DT, name="wsb")
        nc.sync.dma_start(out=wsb, in_=wm_d[:, :])
        bsb = cpool.tile([128, 1], f32, name="bsb")
        nc.sync.dma_start(out=bsb, in_=bias_d[:, :])

        # three persistent rotating window tiles, fully zeroed once; DMAs only
        # rewrite the 64x64 interiors so pads/margins stay zero across reuse.
        def _memset0(ap):
            # Memset of an f32r-typed AP is invalid ISA; zero via an f32 view.
            if DT == mybir.dt.float32r:
                ap = ap.bitcast(f32)
            nc.gpsimd.memset(ap, 0.0)

        xwins = []
        for i in range(3):
            t = xpool.tile([96, XFREE], DT, name=f"xwin{i}", tag=f"xwin{i}")
            _memset0(t[:, :])
            xwins.append(t)

        for win in range(NWIN):
            d0 = GD * win
            xw = xwins[win % 3]
            # input planes d0-1 .. d0+4 -> partition blocks 16*pl
            lo_pl = 1 if win == 0 else 0
            hi_pl = NPL - 1 if win == NWIN - 1 else NPL
            if win == NWIN - 1:
                # plane-slot 5 (d=64) holds stale data from an earlier window;
                # re-zero quadrant [64:96] BEFORE the pl=4 DMA refills it.
                _memset0(xw[64:96, :])
            dst = xw[16 * lo_pl:16 * hi_pl,
                     MARGIN + PW + 1: MARGIN + PW + 1 + 64 * PW]
            dst = dst.rearrange("p (h w) -> p h w", w=PW)[:, :, 0:W]
            src = xt_d[d0 - 1 + lo_pl: d0 - 1 + hi_pl]
            src = src.rearrange("d c h w -> (d c) h w")
            nc.sync.dma_start(out=dst, in_=src)

            osb = opool.tile([128, D * W], f32, name="osb", tag="osb")
            for r0, nr in ROW_CHUNKS:
                n0 = r0 * PW
                n = nr * PW
                ps = pspool.tile([128, 512], f32, name="ps", tag="ps")
                for i in range(9):
                    kh, kw = divmod(i, 3)
                    delta = (kh - 1) * PW + (kw - 1)
                    rs = MARGIN + n0 + delta
                    lhsT = wsb[:, 128 * i:128 * (i + 1)]
                    rhs = xw[:, rs:rs + n]
                    if MM_CAST is not None:
                        lhsT = lhsT.bitcast(MM_CAST)
                        rhs = rhs.bitcast(MM_CAST)
                    nc.tensor.matmul(
                        ps[:, :n],
                        lhsT=lhsT,
                        rhs=rhs,
                        start=(i == 0),
                        stop=(i == 8),
                    )
                # PSUM -> SBUF: fused bias add + 66->64 row compaction
                pv = ps[:, :n].rearrange("p (r z) -> p r z", z=PW)[:, :, 1:W + 1]
                ov = osb[:, (r0 - 1) * W:(r0 - 1 + nr) * W]
                ov = ov.rearrange("p (r z) -> p r z", z=W)
                nc.scalar.activation(
                    out=ov,
                    in_=pv,
                    func=mybir.ActivationFunctionType.Identity,
                    bias=bsb[:, 0:1],
                    scale=1.0,
                )

            # one contiguous DMA: partition 32j+c -> y[c, d0+j, :, :]
            dst = y_d[:, d0:d0 + GD].rearrange("c j h w -> j c (h w)")
            nc.sync.dma_start(out=dst, in_=osb[:, :])


_NC_CACHE = {}


def _get_nc():
    if "nc" not in _NC_CACHE:
        nc = bacc.Bacc("TRN2", target_bir_lowering=False, debug=False)
        xt_d = nc.dram_tensor("xt", [D, CIN, H, W], DT, kind="ExternalInput")
        wm_d = nc.dram_tensor("wmat", [96, 9 * 128], DT, kind="ExternalInput")
        bias_d = nc.dram_tensor("bias", [128, 1], f32, kind="ExternalInput")
        y_d = nc.dram_tensor("y", [COUT, D, H, W], f32, kind="ExternalOutput")
        with TileContext(nc) as tc:
            _conv_body(tc, xt_d, wm_d, bias_d, y_d)
        nc.finalize()  # runs Bacc regalloc/DCE passes, then freezes
        _NC_CACHE["nc"] = nc
    return _NC_CACHE["nc"]


def _run(inputs, trace=False):
    w, bias = _host_hypernet(inputs)
    x = np.asarray(inputs["x"], np.float32)
    in_maps = []
    for b in range(B):
        xt = np.ascontiguousarray(
            np.transpose(x[b], (1, 0, 2, 3)).astype(NPDT))
        in_maps.append({
            "xt": xt,
            "wmat": _build_wmat(w[b]),
            "bias": np.ascontiguousarray(np.tile(bias[b], GD).reshape(128, 1)),
        })
    nc = _get_nc()
    res = run_bass_kernel_spmd(
        nc, in_maps, core_ids=list(range(N_CORES)), trace=trace,
    )
    y = np.stack([res.results[b]["y"] for b in range(B)])
    return y, res


def kernel(**inputs) -> np.ndarray:
    y, _ = _run(inputs, trace=False)
    return y



# revision 2
# speedup vs baseline: 1.4106x; 1.4106x over previous
"""Trainium2 Bass kernel for the hypernet-Conv3D module.

Strategy (data-parallel over batch, one sample per NeuronCore):
  - The tiny hypernet MLP (~2 MFLOP vs 58 GFLOP for the conv) runs on the
    host in fp32 numpy; it produces per-sample conv weights [32,16,3,3,3] and
    biases [32], repacked into matmul-ready block-Toeplitz layouts (bf16).
  - The 3D conv runs on device as an implicit GEMM ("Toeplitz-D"):
      * x is pre-padded on the host to [66, 16, 66x66] bf16 (zero plane at
        d=-1/64, zero-padded 66x66 hw planes), so each window's input DMA is
        one fully contiguous [96 x 8712B] transfer and the SBUF pads never
        need re-zeroing (only the static margins, memset once at startup).
      * PSUM tile [128 = 4 outplanes x 32 cout, N cols]; 9 accumulating bf16
        matmuls (one per (kh,kw) offset, applied as a free-dim shift of the
        rhs AP). kd offsets ride inside the block-Toeplitz lhsT [96, 128].
      * Chunks are row-aligned (7 padded rows = 462 cols, final 1-row chunk)
        so the ScalarE PSUM->SBUF evacuation (bias fused) also compacts the
        66-wide padded rows to 64-wide valid rows, casting to bf16.
      * Output y is written as bf16 (one contiguous 1 MB DMA per window),
        upcast to f32 on the host.
  - Scheduling: input DMAs trigger from the SP queue, output DMAs from the
    Activation queue, so a blocked output trigger can never head-of-line
    block input prefetch.  A run of small warm-up matmuls bridges the initial
    input-DMA latency so the PE p-state ramp is hot when real work arrives.
"""

import numpy as np
import ml_dtypes

import concourse.bacc as bacc
import concourse.mybir as mybir
from concourse.tile import TileContext
from concourse.bass_utils import run_bass_kernel_spmd

B, CIN, COUT, K = 8, 16, 32, 3
D = H = W = 64
NUM_W = CIN * COUT * K**3  # 13824

PW = W + 2          # 66
P2 = PW * PW        # 4356 padded plane
MARGIN = PW + 1     # 67 >= max |(kh-1)*66 + (kw-1)|
XFREE = P2 + 2 * MARGIN  # 4490
GD = 4              # output d-planes per window
NWIN = D // GD      # 16
NPL = GD + 2        # input planes per window
N_CORES = 8
N_WARMUP = 76       # PE p-state warm-up matmuls (64 cols each)
# (start_padded_row, n_rows) chunks covering padded rows 1..64
ROW_CHUNKS = [(1 + 7 * i, 7) for i in range(9)] + [(64, 1)]

f32 = mybir.dt.float32
bf16 = mybir.dt.bfloat16
DT = bf16
NPDT = ml_dtypes.bfloat16


# ---------------------------------------------------------------- host side

def _host_hypernet(inputs):
    f = np.asarray(inputs["features"], np.float32)
    fc0_w = np.asarray(inputs["fc0_w"], np.float32)
    fc0_b = np.asarray(inputs["fc0_b"], np.float32)
    fc1_w = np.asarray(inputs["fc1_w"], np.float32)
    fc1_b = np.asarray(inputs["fc1_b"], np.float32)
    a0 = np.float32(np.asarray(inputs["a0"]).reshape(-1)[0])
    a1 = np.float32(np.asarray(inputs["a1"]).reshape(-1)[0])
    wg_w = np.asarray(inputs["wg_w"], np.float32)
    wg_b = np.asarray(inputs["wg_b"], np.float32)
    h = f @ fc0_w.T + fc0_b
    h = np.where(h >= 0, h, a0 * h)
    h = h @ fc1_w.T + fc1_b
    h = np.where(h >= 0, h, a1 * h)
    params = h @ wg_w.T + wg_b
    w = params[:, :NUM_W].reshape(B, COUT, CIN, K, K, K).astype(np.float32)
    bias = params[:, NUM_W:].astype(np.float32)
    return w, bias


def _build_wmat(w):
    """w: [32,16,3,3,3] -> [96, 9*128] block-Toeplitz lhsT bank (bf16).

    Column block i = 3*kh + kw holds lhsT_i with
      lhsT_i[16*pl + cin, 32*j + c] = w[c, cin, pl - j, kh, kw]  (0 <= pl-j <= 2)
    """
    wmat = np.zeros((9, 96, 128), np.float32)
    wt = np.transpose(w, (3, 4, 1, 0, 2))  # [kh, kw, cin, cout, kd]
    for kh in range(3):
        for kw in range(3):
            i = 3 * kh + kw
            for j in range(GD):
                for kd in range(3):
                    pl = j + kd
                    wmat[i, 16 * pl:16 * pl + 16, 32 * j:32 * j + 32] = \
                        wt[kh, kw, :, :, kd]
    return np.ascontiguousarray(
        wmat.transpose(1, 0, 2).reshape(96, 9 * 128).astype(NPDT))


def _build_xpad(xb):
    """xb: [16, 64, 64, 64] f32 -> [66, 16, 4356] bf16 pre-padded planes.

    Plane index p holds depth d = p - 1; p = 0 and p = 65 are all-zero
    (conv zero padding in d).  Each [16, 4356] plane is the 66x66 zero-
    padded hw plane, flattened.
    """
    xp = np.zeros((D + 2, CIN, PW, PW), NPDT)
    xp[1:D + 1, :, 1:H + 1, 1:W + 1] = np.transpose(
        xb, (1, 0, 2, 3)).astype(NPDT)
    return np.ascontiguousarray(xp.reshape(D + 2, CIN, P2))


# -------------------------------------------------------------- device side

def _conv_body(tc, xt_d, wm_d, bias_d, y_d):
    nc = tc.nc
    with (
        tc.tile_pool(name="const", bufs=1) as cpool,
        tc.tile_pool(name="xw", bufs=1) as xpool,
        tc.tile_pool(name="osb", bufs=3) as opool,
        tc.tile_pool(name="ps", bufs=6, space="PSUM") as pspool,
        tc.tile_pool(name="psw", bufs=1, space="PSUM") as pswpool,
    ):
        wsb = cpool.tile([96, 9 * 128], DT, name="wsb")
        nc.sync.dma_start(out=wsb, in_=wm_d[:, :])
        bsb = cpool.tile([128, 1], f32, name="bsb")
        nc.sync.dma_start(out=bsb, in_=bias_d[:, :])

        # three rotating window tiles; only the static head/tail margins need
        # zeroing (every window DMA rewrites the full 4356-elem plane region,
        # pads included, from the pre-padded DRAM copy).
        xwins = []
        for i in range(3):
            t = xpool.tile([96, XFREE], DT, name=f"xwin{i}", tag=f"xwin{i}")
            nc.gpsimd.memset(t[:, 0:MARGIN], 0.0)
            nc.gpsimd.memset(t[:, MARGIN + P2:XFREE], 0.0)
            xwins.append(t)

        # PE p-state warm-up: small matmuls on the weight bank keep the
        # tensor engine continuously busy through its frequency ramp while
        # the first input window is still in flight.
        psw = pswpool.tile([128, 64], f32, name="psw", tag="psw")
        for _ in range(N_WARMUP):
            nc.tensor.matmul(
                psw[:, :], lhsT=wsb[:, 0:128], rhs=wsb[:, 0:64],
                start=True, stop=True,
            )

        for win in range(NWIN):
            d0 = GD * win
            xw = xwins[win % 3]
            # input planes d0-1 .. d0+4 live at xt_d[d0 .. d0+6] (pre-padded
            # d axis is shifted by one and bracketed by zero planes).
            dst = xw[:, MARGIN:MARGIN + P2]
            src = xt_d[d0:d0 + NPL].rearrange("d c n -> (d c) n")
            nc.sync.dma_start(out=dst, in_=src)

            osb = opool.tile([128, D * W], DT, name="osb", tag="osb")
            for r0, nr in ROW_CHUNKS:
                n0 = r0 * PW
                n = nr * PW
                ps = pspool.tile([128, 512], f32, name="ps", tag="ps")
                for i in range(9):
                    kh, kw = divmod(i, 3)
                    delta = (kh - 1) * PW + (kw - 1)
                    rs = MARGIN + n0 + delta
                    nc.tensor.matmul(
                        ps[:, :n],
                        lhsT=wsb[:, 128 * i:128 * (i + 1)],
                        rhs=xw[:, rs:rs + n],
                        start=(i == 0),
                        stop=(i == 8),
                    )
                # PSUM -> SBUF: fused bias add + 66->64 row compaction + bf16
                pv = ps[:, :n].rearrange("p (r z) -> p r z", z=PW)[:, :, 1:W + 1]
                ov = osb[:, (r0 - 1) * W:(r0 - 1 + nr) * W]
                ov = ov.rearrange("p (r z) -> p r z", z=W)
                nc.scalar.activation(
                    out=ov,
                    in_=pv,
                    func=mybir.ActivationFunctionType.Identity,
                    bias=bsb[:, 0:1],
                    scale=1.0,
                )

            # one contiguous DMA from the Activation queue so it can never
            # block input prefetch on the SP queue:
            # partition 32j+c -> y[c, d0+j, :, :]
            dst = y_d[:, d0:d0 + GD].rearrange("c j h w -> j c (h w)")
            nc.scalar.dma_start(out=dst, in_=osb[:, :])


_NC_CACHE = {}


def _get_nc():
    if "nc" not in _NC_CACHE:
        nc = bacc.Bacc("TRN2", target_bir_lowering=False, debug=False)
        xt_d = nc.dram_tensor("xt", [D + 2, CIN, P2], DT, kind="ExternalInput")
        wm_d = nc.dram_tensor("wmat", [96, 9 * 128], DT, kind="ExternalInput")
        bias_d = nc.dram_tensor("bias", [128, 1], f32, kind="ExternalInput")
        y_d = nc.dram_tensor("y", [COUT, D, H, W], DT, kind="ExternalOutput")
        with TileContext(nc) as tc:
            _conv_body(tc, xt_d, wm_d, bias_d, y_d)
        nc.finalize()  # runs Bacc regalloc/DCE passes, then freezes
        _NC_CACHE["nc"] = nc
    return _NC_CACHE["nc"]


def _run(inputs, trace=False):
    w, bias = _host_hypernet(inputs)
    x = np.asarray(inputs["x"], np.float32)
    in_maps = []
    for b in range(B):
        in_maps.append({
            "xt": _build_xpad(x[b]),
            "wmat": _build_wmat(w[b]),
            "bias": np.ascontiguousarray(np.tile(bias[b], GD).reshape(128, 1)),
        })
    nc = _get_nc()
    res = run_bass_kernel_spmd(
        nc, in_maps, core_ids=list(range(N_CORES)), trace=trace,
    )
    y = np.stack([res.results[b]["y"].astype(np.float32) for b in range(B)])
    return y, res


def kernel(**inputs) -> np.ndarray:
    y, _ = _run(inputs, trace=False)
    return y


# revision 6
# speedup vs baseline: 1.4191x; 1.0060x over previous
"""Trainium2 Bass kernel for the hypernet-Conv3D module.

Strategy (data-parallel over batch, one sample per NeuronCore):
  - The tiny hypernet MLP (~2 MFLOP vs 58 GFLOP for the conv) runs on the
    host in fp32 numpy; it produces per-sample conv weights [32,16,3,3,3] and
    biases [32], repacked into matmul-ready block-Toeplitz layouts (bf16).
  - The 3D conv runs on device as an implicit GEMM ("Toeplitz-D"):
      * x is pre-padded on the host to [66, 16, 66x66] bf16 (zero plane at
        d=-1/64, zero-padded 66x66 hw planes), so each window's input DMA is
        one fully contiguous [96 x 8712B] transfer and the SBUF pads never
        need re-zeroing (only the static margins, memset once at startup).
      * PSUM tile [128 = 4 outplanes x 32 cout, N cols]; 9 accumulating bf16
        matmuls (one per (kh,kw) offset, applied as a free-dim shift of the
        rhs AP). kd offsets ride inside the block-Toeplitz lhsT [96, 128].
      * Chunks are row-aligned (7 padded rows = 462 cols, final 1-row chunk)
        so the ScalarE PSUM->SBUF evacuation (bias fused) also compacts the
        66-wide padded rows to 64-wide valid rows, casting to bf16.
      * Output y is written as bf16 (one contiguous 1 MB DMA per window),
        upcast to f32 on the host.
  - Scheduling: input DMAs trigger from the SP queue, output DMAs from the
    Activation queue, so a blocked output trigger can never head-of-line
    block input prefetch.  A run of small warm-up matmuls bridges the initial
    input-DMA latency so the PE p-state ramp is hot when real work arrives.
"""

import numpy as np
import ml_dtypes

import concourse.bacc as bacc
import concourse.mybir as mybir
from concourse.tile import TileContext
from concourse.bass_utils import run_bass_kernel_spmd

B, CIN, COUT, K = 8, 16, 32, 3
D = H = W = 64
NUM_W = CIN * COUT * K**3  # 13824

PW = W + 2          # 66
P2 = PW * PW        # 4356 padded plane
MARGIN = PW + 1     # 67 >= max |(kh-1)*66 + (kw-1)|
XFREE = P2 + 2 * MARGIN  # 4490
GD = 4              # output d-planes per window
NWIN = D // GD      # 16
NPL = GD + 2        # input planes per window
N_CORES = 8
N_WARMUP = 52       # PE p-state warm-up matmuls (64 cols each)
# (start_padded_row, n_rows) chunks covering padded rows 1..64
ROW_CHUNKS = [(1 + 7 * i, 7) for i in range(9)] + [(64, 1)]

f32 = mybir.dt.float32
bf16 = mybir.dt.bfloat16
DT = bf16
NPDT = ml_dtypes.bfloat16


# ---------------------------------------------------------------- host side

def _host_hypernet(inputs):
    f = np.asarray(inputs["features"], np.float32)
    fc0_w = np.asarray(inputs["fc0_w"], np.float32)
    fc0_b = np.asarray(inputs["fc0_b"], np.float32)
    fc1_w = np.asarray(inputs["fc1_w"], np.float32)
    fc1_b = np.asarray(inputs["fc1_b"], np.float32)
    a0 = np.float32(np.asarray(inputs["a0"]).reshape(-1)[0])
    a1 = np.float32(np.asarray(inputs["a1"]).reshape(-1)[0])
    wg_w = np.asarray(inputs["wg_w"], np.float32)
    wg_b = np.asarray(inputs["wg_b"], np.float32)
    h = f @ fc0_w.T + fc0_b
    h = np.where(h >= 0, h, a0 * h)
    h = h @ fc1_w.T + fc1_b
    h = np.where(h >= 0, h, a1 * h)
    params = h @ wg_w.T + wg_b
    w = params[:, :NUM_W].reshape(B, COUT, CIN, K, K, K).astype(np.float32)
    bias = params[:, NUM_W:].astype(np.float32)
    return w, bias


def _build_wmat(w):
    """w: [32,16,3,3,3] -> [96, 9*128] block-Toeplitz lhsT bank (bf16).

    Column block i = 3*kh + kw holds lhsT_i with
      lhsT_i[16*pl + cin, 32*j + c] = w[c, cin, pl - j, kh, kw]  (0 <= pl-j <= 2)
    """
    wmat = np.zeros((9, 96, 128), np.float32)
    wt = np.transpose(w, (3, 4, 1, 0, 2))  # [kh, kw, cin, cout, kd]
    for kh in range(3):
        for kw in range(3):
            i = 3 * kh + kw
            for j in range(GD):
                for kd in range(3):
                    pl = j + kd
                    wmat[i, 16 * pl:16 * pl + 16, 32 * j:32 * j + 32] = \
                        wt[kh, kw, :, :, kd]
    return np.ascontiguousarray(
        wmat.transpose(1, 0, 2).reshape(96, 9 * 128).astype(NPDT))


def _build_xpad(xb):
    """xb: [16, 64, 64, 64] f32 -> [66, 16, 4356] bf16 pre-padded planes.

    Plane index p holds depth d = p - 1; p = 0 and p = 65 are all-zero
    (conv zero padding in d).  Each [16, 4356] plane is the 66x66 zero-
    padded hw plane, flattened.
    """
    xp = np.zeros((D + 2, CIN, PW, PW), NPDT)
    xp[1:D + 1, :, 1:H + 1, 1:W + 1] = np.transpose(
        xb, (1, 0, 2, 3)).astype(NPDT)
    return np.ascontiguousarray(xp.reshape(D + 2, CIN, P2))


# -------------------------------------------------------------- device side

def _conv_body(tc, xt_d, wm_d, bias_d, y_d):
    nc = tc.nc
    with (
        tc.tile_pool(name="const", bufs=1) as cpool,
        tc.tile_pool(name="xw", bufs=1) as xpool,
        tc.tile_pool(name="osb", bufs=3) as opool,
        tc.tile_pool(name="ps", bufs=6, space="PSUM") as pspool,
        tc.tile_pool(name="psw", bufs=1, space="PSUM") as pswpool,
    ):
        wsb = cpool.tile([96, 9 * 128], DT, name="wsb")
        nc.sync.dma_start(out=wsb, in_=wm_d[:, :])

        # three rotating window tiles; only the static head/tail margins need
        # zeroing (every window DMA rewrites the full 4356-elem plane region,
        # pads included, from the pre-padded DRAM copy).
        xwins = []
        for i in range(3):
            t = xpool.tile([96, XFREE], DT, name=f"xwin{i}", tag=f"xwin{i}")
            nc.gpsimd.memset(t[:, 0:MARGIN], 0.0)
            nc.gpsimd.memset(t[:, MARGIN + P2:XFREE], 0.0)
            xwins.append(t)

        # PE p-state warm-up: small matmuls on the weight bank keep the
        # tensor engine continuously busy through its frequency ramp while
        # the first input window is still in flight.
        psw = pswpool.tile([128, 64], f32, name="psw", tag="psw")
        for _ in range(N_WARMUP):
            nc.tensor.matmul(
                psw[:, :], lhsT=wsb[:, 0:128], rhs=wsb[:, 0:64],
                start=True, stop=True,
            )

        for win in range(NWIN):
            d0 = GD * win
            xw = xwins[win % 3]
            # input planes d0-1 .. d0+4 live at xt_d[d0 .. d0+6] (pre-padded
            # d axis is shifted by one and bracketed by zero planes).
            dst = xw[:, MARGIN:MARGIN + P2]
            src = xt_d[d0:d0 + NPL].rearrange("d c n -> (d c) n")
            nc.sync.dma_start(out=dst, in_=src)
            if win == 0:
                # bias load sits behind the window-0 input on the SP queue so
                # it cannot delay the critical-path transfer; it is only
                # needed by the first PSUM evacuation, ~3 us later.
                bsb = cpool.tile([128, 1], f32, name="bsb")
                nc.sync.dma_start(out=bsb, in_=bias_d[:, :])

            osb = opool.tile([128, D * W], DT, name="osb", tag="osb")
            for r0, nr in ROW_CHUNKS:
                n0 = r0 * PW
                n = nr * PW
                ps = pspool.tile([128, 512], f32, name="ps", tag="ps")
                for i in range(9):
                    kh, kw = divmod(i, 3)
                    delta = (kh - 1) * PW + (kw - 1)
                    rs = MARGIN + n0 + delta
                    nc.tensor.matmul(
                        ps[:, :n],
                        lhsT=wsb[:, 128 * i:128 * (i + 1)],
                        rhs=xw[:, rs:rs + n],
                        start=(i == 0),
                        stop=(i == 8),
                    )
                # PSUM -> SBUF: fused bias add + 66->64 row compaction + bf16
                pv = ps[:, :n].rearrange("p (r z) -> p r z", z=PW)[:, :, 1:W + 1]
                ov = osb[:, (r0 - 1) * W:(r0 - 1 + nr) * W]
                ov = ov.rearrange("p (r z) -> p r z", z=W)
                nc.scalar.activation(
                    out=ov,
                    in_=pv,
                    func=mybir.ActivationFunctionType.Identity,
                    bias=bsb[:, 0:1],
                    scale=1.0,
                )
                if win == NWIN - 1:
                    # last window: drain each chunk as soon as it is
                    # evacuated to shorten the end-of-kernel tail.
                    dst = y_d[:, d0:d0 + GD, r0 - 1:r0 - 1 + nr]
                    dst = dst.rearrange("c j h w -> j c (h w)")
                    nc.scalar.dma_start(
                        out=dst, in_=osb[:, (r0 - 1) * W:(r0 - 1 + nr) * W])

            if win < NWIN - 1:
                # one contiguous DMA from the Activation queue so it can
                # never block input prefetch on the SP queue:
                # partition 32j+c -> y[c, d0+j, :, :]
                dst = y_d[:, d0:d0 + GD].rearrange("c j h w -> j c (h w)")
                nc.scalar.dma_start(out=dst, in_=osb[:, :])


_NC_CACHE = {}


def _get_nc():
    if "nc" not in _NC_CACHE:
        nc = bacc.Bacc("TRN2", target_bir_lowering=False, debug=False)
        xt_d = nc.dram_tensor("xt", [D + 2, CIN, P2], DT, kind="ExternalInput")
        wm_d = nc.dram_tensor("wmat", [96, 9 * 128], DT, kind="ExternalInput")
        bias_d = nc.dram_tensor("bias", [128, 1], f32, kind="ExternalInput")
        y_d = nc.dram_tensor("y", [COUT, D, H, W], DT, kind="ExternalOutput")
        with TileContext(nc) as tc:
            _conv_body(tc, xt_d, wm_d, bias_d, y_d)
        nc.finalize()  # runs Bacc regalloc/DCE passes, then freezes
        _NC_CACHE["nc"] = nc
    return _NC_CACHE["nc"]


def _run(inputs, trace=False):
    w, bias = _host_hypernet(inputs)
    x = np.asarray(inputs["x"], np.float32)
    in_maps = []
    for b in range(B):
        in_maps.append({
            "xt": _build_xpad(x[b]),
            "wmat": _build_wmat(w[b]),
            "bias": np.ascontiguousarray(np.tile(bias[b], GD).reshape(128, 1)),
        })
    nc = _get_nc()
    res = run_bass_kernel_spmd(
        nc, in_maps, core_ids=list(range(N_CORES)), trace=trace,
    )
    y = np.stack([res.results[b]["y"].astype(np.float32) for b in range(B)])
    return y, res


def kernel(**inputs) -> np.ndarray:
    y, _ = _run(inputs, trace=False)
    return y


# revision 10
# speedup vs baseline: 1.5564x; 1.0968x over previous
"""Trainium2 Bass kernel for the hypernet-Conv3D module.

Strategy (data-parallel over batch, one sample per NeuronCore):
  - The tiny hypernet MLP (~2 MFLOP vs 58 GFLOP for the conv) runs on the
    host in fp32 numpy; it produces per-sample conv weights [32,16,3,3,3] and
    biases [32], repacked into matmul-ready block-Toeplitz layouts (bf16).
  - The 3D conv runs on device as an implicit GEMM.  Per matmul:
      * contraction partitions p = 24*s + 8*kh + c8 (s: 5 depth slots, kh: 3
        row-shifted plane copies prepared on the host, c8: 8 of the 16 input
        channels -- the two channel halves sit side by side in the free dim);
      * output partitions 32*j + c (j: 3 depth planes via the block-Toeplitz
        kd band inside the lhsT, c: 32 output channels);
      * so only 6 accumulating matmuls (2 channel halves x 3 kw column
        shifts) cover all 432 contraction terms for 3 output planes -- 2.0
        streamed PE columns per output position instead of 9/4 = 2.25.
  - x is pre-padded and pre-shifted on the host to [68, 3, 8, 2, 4356] bf16
    (zero planes bracketing d, 66x66 zero-padded hw planes, 3 row shifts),
    so each window's input DMA is one fully contiguous [120 x 17424B]
    transfer and SBUF needs no zero maintenance at all.
  - PSUM chunks of 7 padded rows (462 cols); the ScalarE PSUM->SBUF
    evacuation (bias fused) compacts 66-wide padded rows to 64-wide valid
    rows, casting to bf16.  Output y is written as bf16 (one contiguous
    0.75 MB DMA per window), upcast to f32 on the host.
  - Scheduling: input DMAs trigger from the SP queue, output DMAs from the
    Activation queue, so a blocked output trigger can never head-of-line
    block input prefetch.  A run of small warm-up matmuls bridges the initial
    input-DMA latency so the PE p-state ramp is hot when real work arrives;
    the first window's input lands in two row-split DMAs so its first chunks
    arrive early.  The last window drains per-chunk to shorten the tail.
"""

import numpy as np
import ml_dtypes

import concourse.bacc as bacc
import concourse.mybir as mybir
from concourse.tile import TileContext
from concourse.bass_utils import run_bass_kernel_spmd

B, CIN, COUT, K = 8, 16, 32, 3
D = H = W = 64
NUM_W = CIN * COUT * K**3  # 13824

PW = W + 2          # 66 padded row width
P2 = PW * PW        # 4356 padded plane
GD = 3              # output d-planes per window
NWIN = 22           # ceil(64 / 3); last window emits 1 plane
SLOTS = 5           # input depth slots per window (GD + 2)
NPAD = D + 4        # 68 pre-padded depth planes (zeros at 0, 65, 66, 67)
NPART = 120         # 24 * SLOTS contraction partitions
XFREE = 2 * P2      # two channel halves side by side
N_CORES = 8
N_WARMUP = 56       # PE p-state warm-up matmuls (64 cols each)
W0SPLIT = 29        # first window: DMA rows [0,29) then [29,66)
# (start_padded_row, n_rows) chunks covering padded rows 1..64
ROW_CHUNKS = [(1 + 7 * i, 7) for i in range(9)] + [(64, 1)]

f32 = mybir.dt.float32
bf16 = mybir.dt.bfloat16
DT = bf16
NPDT = ml_dtypes.bfloat16


# ---------------------------------------------------------------- host side

def _host_hypernet(inputs):
    f = np.asarray(inputs["features"], np.float32)
    fc0_w = np.asarray(inputs["fc0_w"], np.float32)
    fc0_b = np.asarray(inputs["fc0_b"], np.float32)
    fc1_w = np.asarray(inputs["fc1_w"], np.float32)
    fc1_b = np.asarray(inputs["fc1_b"], np.float32)
    a0 = np.float32(np.asarray(inputs["a0"]).reshape(-1)[0])
    a1 = np.float32(np.asarray(inputs["a1"]).reshape(-1)[0])
    wg_w = np.asarray(inputs["wg_w"], np.float32)
    wg_b = np.asarray(inputs["wg_b"], np.float32)
    h = f @ fc0_w.T + fc0_b
    h = np.where(h >= 0, h, a0 * h)
    h = h @ fc1_w.T + fc1_b
    h = np.where(h >= 0, h, a1 * h)
    params = h @ wg_w.T + wg_b
    w = params[:, :NUM_W].reshape(B, COUT, CIN, K, K, K).astype(np.float32)
    bias = params[:, NUM_W:].astype(np.float32)
    return w, bias


def _build_wmat(w):
    """w: [32,16,3,3,3] -> [120, 6*96] lhsT bank (bf16).

    Column bank b = 3*g + kw holds lhsT_b with
      lhsT_b[24*s + 8*kh + c8, 32*j + c] = w[c, 8*g + c8, s - j, kh, kw]
    for 0 <= s - j <= 2, zero elsewhere (block-Toeplitz kd band).
    """
    wmat = np.zeros((SLOTS, 3, 8, 2, 3, GD, COUT), np.float32)
    # wt[kh, kw, kd, cin, cout]
    wt = np.transpose(w, (3, 4, 2, 1, 0))
    for s in range(SLOTS):
        for j in range(GD):
            kd = s - j
            if 0 <= kd <= 2:
                # [kh, kw, cin, cout] -> [kh, c8, g, kw]...
                blk = wt[:, :, kd]  # [kh, kw, 16, 32]
                blk = blk.reshape(3, 3, 2, 8, COUT)  # [kh, kw, g, c8, cout]
                wmat[s, :, :, :, :, j, :] = np.transpose(blk, (0, 3, 2, 1, 4))
    return np.ascontiguousarray(
        wmat.reshape(NPART, 6 * 96).astype(NPDT))


def _build_xpad(xb):
    """xb: [16, 64, 64, 64] f32 -> [68, 3, 8, 2, 4356] bf16.

    Plane index pd holds depth d = pd - 1; pd = 0 and pd >= 65 are all-zero.
    Copy kh holds the 66x66 zero-padded hw plane shifted so padded row r
    contains plain padded row r + kh - 1 (zeros shifted in at the edges).
    Channel cin = 8*g + c8 lives at [pd, kh, c8, g, :].
    """
    bp = np.zeros((D, 8, 2, PW, PW), NPDT)  # [d, c8, g, 66, 66]
    # xb is [cin, d, h, w]; want [d, c8, g, h, w] with cin = 8g + c8
    xs = np.transpose(xb.astype(NPDT).reshape(2, 8, D, H, W),
                      (2, 1, 0, 3, 4))  # [d, c8, g, h, w]
    bp[:, :, :, 1:H + 1, 1:W + 1] = xs
    xp = np.zeros((NPAD, 3, 8, 2, PW, PW), NPDT)
    for kh in range(3):
        sh = kh - 1
        rlo, rhi = max(0, -sh), min(PW, PW - sh)
        xp[1:D + 1, kh, :, :, rlo:rhi] = np.transpose(
            bp[:, :, :, rlo + sh:rhi + sh], (0, 1, 2, 3, 4))
    return np.ascontiguousarray(xp.reshape(NPAD, 3, 8, 2, P2))


# -------------------------------------------------------------- device side

def _conv_body(tc, xt_d, wm_d, bias_d, y_d):
    nc = tc.nc
    with (
        tc.tile_pool(name="const", bufs=1) as cpool,
        tc.tile_pool(name="xw", bufs=1) as xpool,
        tc.tile_pool(name="osb", bufs=3) as opool,
        tc.tile_pool(name="ps", bufs=6, space="PSUM") as pspool,
        tc.tile_pool(name="psw", bufs=1, space="PSUM") as pswpool,
    ):
        wsb = cpool.tile([NPART, 6 * 96], DT, name="wsb")
        nc.sync.dma_start(out=wsb, in_=wm_d[:, :])

        xwins = [
            xpool.tile([NPART, XFREE], DT, name=f"xwin{i}", tag=f"xwin{i}")
            for i in range(3)
        ]

        # PE p-state warm-up: small matmuls on the weight bank keep the
        # tensor engine continuously busy through its frequency ramp while
        # the first input window is still in flight.
        psw = pswpool.tile([96, 64], f32, name="psw", tag="psw")
        for _ in range(N_WARMUP):
            nc.tensor.matmul(
                psw[:, :], lhsT=wsb[:, 0:96], rhs=wsb[:, 0:64],
                start=True, stop=True,
            )

        for win in range(NWIN):
            d0 = GD * win
            xw = xwins[win % 3]
            # depth slots d0-1 .. d0+3 live at xt_d[d0 .. d0+5) (pre-padded
            # d axis is shifted by one and bracketed by zero planes).
            src = xt_d[d0:d0 + SLOTS].rearrange("d kh c g n -> (d kh c) (g n)")
            if win == 0:
                # split so the first chunks' rows land early
                ncol = W0SPLIT * PW
                dstv = xw[:, :].rearrange("p (g n) -> p g n", g=2)
                srcv = src.rearrange("p (g n) -> p g n", g=2)
                nc.sync.dma_start(out=dstv[:, :, 0:ncol], in_=srcv[:, :, 0:ncol])
                # bias load slots behind the urgent rows on the SP queue; it
                # is only needed by the first PSUM evacuation, ~3 us later.
                bsb = cpool.tile([96, 1], f32, name="bsb")
                nc.sync.dma_start(out=bsb, in_=bias_d[:, :])
                nc.sync.dma_start(out=dstv[:, :, ncol:P2], in_=srcv[:, :, ncol:P2])
            else:
                nc.sync.dma_start(out=xw[:, :], in_=src)

            osb = opool.tile([96, D * W], DT, name="osb", tag="osb")
            for r0, nr in ROW_CHUNKS:
                n0 = r0 * PW
                n = nr * PW
                ps = pspool.tile([96, 512], f32, name="ps", tag="ps")
                for i in range(6):
                    g, kw = divmod(i, 3)
                    rs = g * P2 + n0 + (kw - 1)
                    nc.tensor.matmul(
                        ps[:, :n],
                        lhsT=wsb[:, 96 * i:96 * (i + 1)],
                        rhs=xw[:, rs:rs + n],
                        start=(i == 0),
                        stop=(i == 5),
                    )
                # PSUM -> SBUF: fused bias add + 66->64 row compaction + bf16
                pv = ps[:, :n].rearrange("p (r z) -> p r z", z=PW)[:, :, 1:W + 1]
                ov = osb[:, (r0 - 1) * W:(r0 - 1 + nr) * W]
                ov = ov.rearrange("p (r z) -> p r z", z=W)
                nc.scalar.activation(
                    out=ov,
                    in_=pv,
                    func=mybir.ActivationFunctionType.Identity,
                    bias=bsb[:, 0:1],
                    scale=1.0,
                )
                if win == NWIN - 1:
                    # last window (single valid plane): drain each chunk as
                    # soon as it is evacuated to shorten the tail.
                    dst = y_d[:, d0:d0 + 1, r0 - 1:r0 - 1 + nr]
                    dst = dst.rearrange("c j h w -> (j c) (h w)")
                    nc.scalar.dma_start(
                        out=dst, in_=osb[0:32, (r0 - 1) * W:(r0 - 1 + nr) * W])

            if win < NWIN - 1:
                # one contiguous DMA from the Activation queue so it can
                # never block input prefetch on the SP queue:
                # partition 32j+c -> y[c, d0+j, :, :]
                dst = y_d[:, d0:d0 + GD].rearrange("c j h w -> j c (h w)")
                nc.scalar.dma_start(out=dst, in_=osb[:, :])


_NC_CACHE = {}


def _get_nc():
    if "nc" not in _NC_CACHE:
        nc = bacc.Bacc("TRN2", target_bir_lowering=False, debug=False)
        xt_d = nc.dram_tensor(
            "xt", [NPAD, 3, 8, 2, P2], DT, kind="ExternalInput")
        wm_d = nc.dram_tensor("wmat", [NPART, 6 * 96], DT, kind="ExternalInput")
        bias_d = nc.dram_tensor("bias", [96, 1], f32, kind="ExternalInput")
        y_d = nc.dram_tensor("y", [COUT, D, H, W], DT, kind="ExternalOutput")
        with TileContext(nc) as tc:
            _conv_body(tc, xt_d, wm_d, bias_d, y_d)
        nc.finalize()  # runs Bacc regalloc/DCE passes, then freezes
        _NC_CACHE["nc"] = nc
    return _NC_CACHE["nc"]


def _run(inputs, trace=False):
    w, bias = _host_hypernet(inputs)
    x = np.asarray(inputs["x"], np.float32)
    in_maps = []
    for b in range(B):
        in_maps.append({
            "xt": _build_xpad(x[b]),
            "wmat": _build_wmat(w[b]),
            "bias": np.ascontiguousarray(np.tile(bias[b], GD).reshape(96, 1)),
        })
    nc = _get_nc()
    res = run_bass_kernel_spmd(
        nc, in_maps, core_ids=list(range(N_CORES)), trace=trace,
    )
    y = np.stack([res.results[b]["y"].astype(np.float32) for b in range(B)])
    return y, res


def kernel(**inputs) -> np.ndarray:
    y, _ = _run(inputs, trace=False)
    return y
